# revision 23
# baseline (speedup 1.0000x reference)
"""Trainium2 Bass kernel for nn_LinearCondensed.

Computes out[b, o] = sum_k weight[o, k] * x[b, indx_seqs[o, k]] + bias[o]
with B=2048, IN_F=OUT_F=4096, FAN_IN=32.

Strategy: the gather has no fast on-chip primitive (any materialized gather
moves 32x the data of x itself), so we densify the sparse weight matrix on
the host -- W'[o, i] = sum_{k: indx_seqs[o,k]==i} weight[o, k] -- and run a
dense matmul out = x @ W'^T + bias on the PE array. Operands are cast to
fp16 on the host: the PE streams fp16 at the same 1 cycle/row as fp32r
(bf16 speed) but DMA traffic halves (24MB/core vs 44MB), making the kernel
PE-bound (~110us of matmul) instead of DMA-bound. fp16 products are exact
in the PE's fp22 pipeline and accumulation is fp32, so max rel err stays
at ~4e-4. OUT_F is sharded 8 ways across cores (512 columns each), x is
replicated, bias is added in the PSUM drain.

Pipeline: x loads ride the sync HWDGE queue, weights + bias ride the
scalar HW queue in parallel (gpsimd/vector queues are software-DGE at
~35GB/s), so the first matmul's dependencies (x0 k-tiles 0-3 + W' k-tile
0, ~384KB) land ~10us in, instead of waiting behind 2.5MB on one queue.
A few dummy matmuls on a zeroed scratch tile warm the PE p-state while
those DMAs are in flight. Phase 1 runs b-tiles 0..G-1 k-block-interleaved
so the PE consumes W'/x chunks as they stream in; phase 2 streams the
remaining b-tiles k-inner with x prefetched a few tiles deep. Outputs are
stored as fp16 (host upcasts; adds ~1e-4 rel err) on the scalar queue;
the final tile's drain+store is quarter-split across the scalar+sync
queues to shorten the end-of-kernel critical path. Measured ~132us vs
the ~15us of fixed framework overhead (preamble-to-first-DMA, and an
~8us all-semaphore-zeroing epilogue) + ~113us of PE-bound matmul.
"""

import os
import sys
import types

import numpy as np

import concourse.bacc as bacc
import concourse.mybir as mybir
import concourse.tile as tile
from concourse.bass_utils import run_bass_kernel_spmd

B, IN_F, OUT_F, FAN_IN = 2048, 4096, 4096, 32
NCORES = 8
OSH = OUT_F // NCORES          # 512 output features per core
P = 128                        # partitions
BT = B // P                    # 16 batch tiles
KT = IN_F // P                 # 32 contraction tiles
N = OSH                        # 512 moving columns
G = 5                          # phase-1 b-tiles (k-blocked, paced by w stream)

f32 = mybir.dt.float32
f16 = mybir.dt.float16

_cache = {}


def _enable_ntff_hook():
    """Register the ctypes NTFF profile hook (the image's antenv lacks
    axon_hooks); lets trace=True produce a neuron-profile under axon."""
    try:
        from antenv.axon_hooks import get_axon_ntff_profile_hook  # noqa: F401
        return
    except ImportError:
        pass
    try:
        import antenv
        from trn_agent_boot.trn_boot import _ntff_profile_via_ctypes

        mod = types.ModuleType("antenv.axon_hooks")
        holder = [None]
        mod.set_axon_ntff_profile_hook = lambda h: holder.__setitem__(0, h)
        mod.get_axon_ntff_profile_hook = lambda: holder[0]
        antenv.axon_hooks = mod
        sys.modules["antenv.axon_hooks"] = mod
        mod.set_axon_ntff_profile_hook(
            _ntff_profile_via_ctypes("/opt/axon/libaxon_pjrt.so"))
        import concourse.bass_utils as bu
        bu.upload_artifacts = lambda tmpdir: str(tmpdir)
    except Exception:
        pass


def _build():
    nc = bacc.Bacc()
    # Layouts (host-pretiled, all contiguous):
    #   XT[t, p, a, c] = x[t*128 + c, a*128 + p]   -> per b-tile t: [128, KT*128]
    #   WT[p, a, n]    = W'[o0 + n, a*128 + p]     -> [128, KT, 512]
    XT = nc.declare_dram_parameter("XT", [BT, P, KT * P], f16, isOutput=False)
    WT = nc.declare_dram_parameter("WT", [P, KT, N], f16, isOutput=False)
    BIAS = nc.declare_dram_parameter("BIAS", [P, N], f32, isOutput=False)
    OUT = nc.declare_dram_parameter("OUT", [B, N], f16, isOutput=True)

    XTv = XT.ap().rearrange("t p (a c) -> t p a c", a=KT)

    # k-tile chunking of the early loads: the first matmuls only need
    # x0[k 0:4] + W'[k 0:1], so those land first; later chunks are ordered
    # by the block-interleaved phase-1 consumption order.
    X_CHUNKS = [(0, 4), (4, 4), (8, 8), (16, 8), (24, 8)]
    W_CHUNKS = [(0, 1), (1, 1), (2, 2), (4, 2), (6, 2), (8, 4), (12, 4),
                (16, 4), (20, 4), (24, 4), (28, 4)]
    # phase-1 k-blocks: within a block, all G b-tiles run k-inner, so the
    # first matmuls depend on one small x chunk instead of chunk 0 of every
    # phase-1 tile (the early DMA chain is latency-bound at ~2us/chunk).
    K_BLOCKS = [(0, 4), (4, 4), (8, 8), (16, 8), (24, 8)]

    with tile.TileContext(nc) as tc:
        with (
            tc.tile_pool(name="wpool", bufs=1) as wpool,
            tc.tile_pool(name="xpool", bufs=6) as xpool,
            tc.tile_pool(name="cpool", bufs=1) as cpool,
            tc.tile_pool(name="opool", bufs=3) as opool,
            tc.tile_pool(name="psum", bufs=6, space="PSUM") as psum,
        ):
            xtiles = {}

            def alloc_x(t):
                xs = xpool.tile([P, KT, P], f16, tag="xs")
                xtiles[t] = xs
                return xs

            wsb = wpool.tile([P, KT, N], f16)

            # Phase-1 x tiles: interleave per-tile chunks on the sync queue
            # in first-use order (k-block major, t minor).
            for t in range(G):
                alloc_x(t)
            sched = []
            for t in range(G):
                for s, n in X_CHUNKS:
                    sched.append((s, t, n))
            sched.sort()
            for s, t, n in sched:
                nc.sync.dma_start(xtiles[t][:, s:s + n, :], XTv[t][:, s:s + n, :])

            # Weights + bias stream on the scalar HW queue in parallel
            # (the gpsimd/vector queues are software-DGE and crawl at
            # ~35GB/s; only sync+scalar are hardware queues).
            brow = cpool.tile([P, N], f32)
            for s, n in W_CHUNKS:
                nc.scalar.dma_start(wsb[:, s:s + n, :], WT.ap()[:, s:s + n, :])
            nc.scalar.dma_start(brow[:], BIAS[:])
            wtiles = [wsb[:, a, :] for a in range(KT)]

            # Warm-up: dummy matmuls on a zeroed scratch tile while the
            # first input DMAs are in flight (the PE would idle ~4us
            # anyway); ramps the PE out of its cold p-state so the real
            # stream starts at full clock. Results land in a scratch PSUM
            # tile that is never read.
            scratch = cpool.tile([P, N], f16)
            nc.gpsimd.memset(scratch[:], 0)
            warm = psum.tile([P, N], f32, name="warm", tag="acc")
            for _ in range(7):
                nc.tensor.matmul(warm[:], scratch[:, :P], scratch[:],
                                 start=True, stop=True)

            # bias folded into the PSUM drain: osb = acc + bias (bias row
            # pre-replicated across partitions on host). The final tile
            # splits its drain across the scalar+sync queues so the
            # end-of-kernel store is half as long.
            def finish_tile(t, acc, split=False):
                osb = opool.tile([P, N], f16, tag="osb")
                rows = OUT.ap()[t * P:(t + 1) * P, :]
                if split:
                    q = N // 4
                    for j in range(4):
                        sl = slice(j * q, (j + 1) * q)
                        nc.vector.tensor_tensor(
                            osb[:, sl], acc[:, sl], brow[:, sl],
                            mybir.AluOpType.add)
                        eng = nc.scalar if j % 2 == 0 else nc.sync
                        eng.dma_start(rows[:, sl], osb[:, sl])
                else:
                    nc.vector.tensor_tensor(
                        osb[:], acc[:], brow[:], mybir.AluOpType.add)
                    nc.scalar.dma_start(rows, osb[:])

            # Phase 1: b-tiles 0..G-1, k-block-interleaved, so the PE
            # consumes each weight/x chunk as it lands instead of idling
            # through the 4MB weight stream.
            accs = [psum.tile([P, N], f32, name=f"acc{t}", tag="acc")
                    for t in range(G)]
            for s, n in K_BLOCKS:
                for t in range(G):
                    for a in range(s, s + n):
                        nc.tensor.matmul(
                            accs[t][:], xtiles[t][:, a, :], wtiles[a][:],
                            start=(a == 0), stop=(a == KT - 1),
                        )
            for t in range(G):
                finish_tile(t, accs[t])

            # Phase 2: remaining b-tiles, k-inner, x streamed just in time.
            for t in range(G, BT):
                xsb = alloc_x(t)
                nc.sync.dma_start(xsb[:], XTv[t])
                acc = psum.tile([P, N], f32, tag="acc")
                for a in range(KT):
                    nc.tensor.matmul(
                        acc[:],
                        xsb[:, a, :],      # lhsT: [K=128 (i), M=128 (b)]
                        wtiles[a][:],      # rhs:  [K=128 (i), N=512 (o)]
                        start=(a == 0),
                        stop=(a == KT - 1),
                    )
                finish_tile(t, acc, split=(t == BT - 1))

    nc.compile()
    return nc


def kernel(x, weight, bias, indx_seqs):
    x = np.asarray(x, dtype=np.float32)
    weight = np.asarray(weight, dtype=np.float32)
    bias = np.asarray(bias, dtype=np.float32)
    indx_seqs = np.asarray(indx_seqs)

    if "nc" not in _cache:
        _cache["nc"] = _build()
    nc = _cache["nc"]

    # Densify sparse weights: W'[o, i] += weight[o, k] at i = indx_seqs[o, k]
    wd = np.zeros((OUT_F, IN_F), dtype=np.float32)
    np.add.at(wd, (np.arange(OUT_F)[:, None], indx_seqs), weight)

    # Host pre-tiling into SBUF-friendly layouts (cast to fp16 for the PE).
    # XT[t, p, a, c] = x[t*128+c, a*128+p]
    xt = np.ascontiguousarray(
        x.astype(np.float16).reshape(BT, P, KT, P).transpose(0, 3, 2, 1)
    ).reshape(BT, P, KT * P)
    wd16 = wd.astype(np.float16)
    in_maps = []
    for c in range(NCORES):
        wshard = wd16[c * OSH:(c + 1) * OSH]          # (512, 4096) fp16
        # WT[p, a, n] = W'[o0+n, a*128+p]
        wt = np.ascontiguousarray(
            wshard.reshape(OSH, KT, P).transpose(2, 1, 0))
        in_maps.append({
            "XT": xt,
            "WT": wt,
            "BIAS": np.ascontiguousarray(np.broadcast_to(bias[c * OSH:(c + 1) * OSH], (P, N))),
        })

    trace = bool(int(os.environ.get("BASSK_TRACE", "0"))) or bool(
        os.environ.get("BASS_TRACE"))
    if trace:
        _enable_ntff_hook()
    res = run_bass_kernel_spmd(
        nc, in_maps, list(range(NCORES)), trace=trace,
        trace_cores=list(range(NCORES)) if trace else None,
    )
    _cache["last_results"] = res

    out = np.concatenate([res.results[c]["OUT"] for c in range(NCORES)], axis=1)
    return out.astype(np.float32)


# revision 24
# speedup vs baseline: 1.0043x; 1.0043x over previous
"""Trainium2 Bass kernel for nn_LinearCondensed.

Computes out[b, o] = sum_k weight[o, k] * x[b, indx_seqs[o, k]] + bias[o]
with B=2048, IN_F=OUT_F=4096, FAN_IN=32.

Strategy: the gather has no fast on-chip primitive (any materialized gather
moves 32x the data of x itself), so we densify the sparse weight matrix on
the host -- W'[o, i] = sum_{k: indx_seqs[o,k]==i} weight[o, k] -- and run a
dense matmul out = x @ W'^T + bias on the PE array. Operands are cast to
fp16 on the host: the PE streams fp16 at the same 1 cycle/row as fp32r
(bf16 speed) but DMA traffic halves (24MB/core vs 44MB), making the kernel
PE-bound (~110us of matmul) instead of DMA-bound. fp16 products are exact
in the PE's fp22 pipeline and accumulation is fp32, so max rel err stays
at ~4e-4. OUT_F is sharded 8 ways across cores (512 columns each), x is
replicated, bias is added in the PSUM drain.

Pipeline: x loads ride the sync HWDGE queue, weights + bias ride the
scalar HW queue in parallel (gpsimd/vector queues are software-DGE at
~35GB/s), so the first matmul's dependencies (x0 k-tiles 0-3 + W' k-tile
0, ~384KB) land ~10us in, instead of waiting behind 2.5MB on one queue.
A few dummy matmuls on a zeroed scratch tile warm the PE p-state while
those DMAs are in flight. Phase 1 runs b-tiles 0..G-1 k-block-interleaved
so the PE consumes W'/x chunks as they stream in; phase 2 streams the
remaining b-tiles k-inner with x prefetched a few tiles deep. Outputs are
stored as fp16 (host upcasts; adds ~1e-4 rel err) on the scalar queue;
the final tile's drain+store is quarter-split across the scalar+sync
queues to shorten the end-of-kernel critical path. Measured ~132us vs
the ~15us of fixed framework overhead (preamble-to-first-DMA, and an
~8us all-semaphore-zeroing epilogue) + ~113us of PE-bound matmul.
"""

import os
import sys
import types

import numpy as np

import concourse.bacc as bacc
import concourse.mybir as mybir
import concourse.tile as tile
from concourse.bass_utils import run_bass_kernel_spmd

B, IN_F, OUT_F, FAN_IN = 2048, 4096, 4096, 32
NCORES = 8
OSH = OUT_F // NCORES          # 512 output features per core
P = 128                        # partitions
BT = B // P                    # 16 batch tiles
KT = IN_F // P                 # 32 contraction tiles
N = OSH                        # 512 moving columns
G = 6                          # phase-1 b-tiles (k-blocked, paced by w stream)

f32 = mybir.dt.float32
f16 = mybir.dt.float16

_cache = {}


def _enable_ntff_hook():
    """Register the ctypes NTFF profile hook (the image's antenv lacks
    axon_hooks); lets trace=True produce a neuron-profile under axon."""
    try:
        from antenv.axon_hooks import get_axon_ntff_profile_hook  # noqa: F401
        return
    except ImportError:
        pass
    try:
        import antenv
        from trn_agent_boot.trn_boot import _ntff_profile_via_ctypes

        mod = types.ModuleType("antenv.axon_hooks")
        holder = [None]
        mod.set_axon_ntff_profile_hook = lambda h: holder.__setitem__(0, h)
        mod.get_axon_ntff_profile_hook = lambda: holder[0]
        antenv.axon_hooks = mod
        sys.modules["antenv.axon_hooks"] = mod
        mod.set_axon_ntff_profile_hook(
            _ntff_profile_via_ctypes("/opt/axon/libaxon_pjrt.so"))
        import concourse.bass_utils as bu
        bu.upload_artifacts = lambda tmpdir: str(tmpdir)
    except Exception:
        pass


def _build():
    nc = bacc.Bacc()
    # Layouts (host-pretiled, all contiguous):
    #   XT[t, p, a, c] = x[t*128 + c, a*128 + p]   -> per b-tile t: [128, KT*128]
    #   WT[p, a, n]    = W'[o0 + n, a*128 + p]     -> [128, KT, 512]
    XT = nc.declare_dram_parameter("XT", [BT, P, KT * P], f16, isOutput=False)
    WT = nc.declare_dram_parameter("WT", [P, KT, N], f16, isOutput=False)
    BIAS = nc.declare_dram_parameter("BIAS", [P, N], f32, isOutput=False)
    OUT = nc.declare_dram_parameter("OUT", [B, N], f16, isOutput=True)

    XTv = XT.ap().rearrange("t p (a c) -> t p a c", a=KT)

    # k-tile chunking of the early loads: the first matmuls only need
    # x0[k 0:4] + W'[k 0:1], so those land first; later chunks are ordered
    # by the block-interleaved phase-1 consumption order.
    X_CHUNKS = [(0, 2), (2, 2), (4, 4), (8, 8), (16, 8), (24, 8)]
    W_CHUNKS = [(0, 1), (1, 1), (2, 2), (4, 2), (6, 2), (8, 4), (12, 4),
                (16, 4), (20, 4), (24, 4), (28, 4)]
    # phase-1 k-blocks: within a block, all G b-tiles run k-inner, so the
    # first matmuls depend on one small x chunk instead of chunk 0 of every
    # phase-1 tile (the early DMA chain is latency-bound at ~2us/chunk).
    K_BLOCKS = [(0, 2), (2, 2), (4, 4), (8, 8), (16, 8), (24, 8)]

    with tile.TileContext(nc) as tc:
        with (
            tc.tile_pool(name="wpool", bufs=1) as wpool,
            tc.tile_pool(name="xpool", bufs=6) as xpool,
            tc.tile_pool(name="cpool", bufs=1) as cpool,
            tc.tile_pool(name="opool", bufs=3) as opool,
            tc.tile_pool(name="psum", bufs=7, space="PSUM") as psum,
        ):
            xtiles = {}

            def alloc_x(t):
                xs = xpool.tile([P, KT, P], f16, tag="xs")
                xtiles[t] = xs
                return xs

            wsb = wpool.tile([P, KT, N], f16)

            # Phase-1 x tiles: interleave per-tile chunks on the sync queue
            # in first-use order (k-block major, t minor).
            for t in range(G):
                alloc_x(t)
            sched = []
            for t in range(G):
                for s, n in X_CHUNKS:
                    sched.append((s, t, n))
            sched.sort()
            for s, t, n in sched:
                nc.sync.dma_start(xtiles[t][:, s:s + n, :], XTv[t][:, s:s + n, :])

            # Weights + bias stream on the scalar HW queue in parallel
            # (the gpsimd/vector queues are software-DGE and crawl at
            # ~35GB/s; only sync+scalar are hardware queues).
            brow = cpool.tile([P, N], f32)
            for s, n in W_CHUNKS:
                nc.scalar.dma_start(wsb[:, s:s + n, :], WT.ap()[:, s:s + n, :])
            nc.scalar.dma_start(brow[:], BIAS[:])
            wtiles = [wsb[:, a, :] for a in range(KT)]

            # Warm-up: dummy matmuls on a zeroed scratch tile while the
            # first input DMAs are in flight (the PE would idle ~4us
            # anyway); ramps the PE out of its cold p-state so the real
            # stream starts at full clock. Results land in a scratch PSUM
            # tile that is never read.
            scratch = cpool.tile([P, N], f16)
            nc.gpsimd.memset(scratch[:], 0)
            warm = psum.tile([P, N], f32, name="warm", tag="acc")
            for _ in range(7):
                nc.tensor.matmul(warm[:], scratch[:, :P], scratch[:],
                                 start=True, stop=True)

            # bias folded into the PSUM drain: osb = acc + bias (bias row
            # pre-replicated across partitions on host). The final tile
            # splits its drain across the scalar+sync queues so the
            # end-of-kernel store is half as long.
            def finish_tile(t, acc, split=False):
                osb = opool.tile([P, N], f16, tag="osb")
                rows = OUT.ap()[t * P:(t + 1) * P, :]
                if split:
                    q = N // 4
                    for j in range(4):
                        sl = slice(j * q, (j + 1) * q)
                        nc.vector.tensor_tensor(
                            osb[:, sl], acc[:, sl], brow[:, sl],
                            mybir.AluOpType.add)
                        eng = nc.scalar if j % 2 == 0 else nc.sync
                        eng.dma_start(rows[:, sl], osb[:, sl])
                else:
                    nc.vector.tensor_tensor(
                        osb[:], acc[:], brow[:], mybir.AluOpType.add)
                    nc.scalar.dma_start(rows, osb[:])

            # Phase 1: b-tiles 0..G-1, k-block-interleaved, so the PE
            # consumes each weight/x chunk as it lands instead of idling
            # through the 4MB weight stream.
            accs = [psum.tile([P, N], f32, name=f"acc{t}", tag="acc")
                    for t in range(G)]
            for s, n in K_BLOCKS:
                for t in range(G):
                    for a in range(s, s + n):
                        nc.tensor.matmul(
                            accs[t][:], xtiles[t][:, a, :], wtiles[a][:],
                            start=(a == 0), stop=(a == KT - 1),
                        )
            for t in range(G):
                finish_tile(t, accs[t])

            # Phase 2: remaining b-tiles, k-inner, x streamed just in time.
            for t in range(G, BT):
                xsb = alloc_x(t)
                nc.sync.dma_start(xsb[:], XTv[t])
                acc = psum.tile([P, N], f32, tag="acc")
                for a in range(KT):
                    nc.tensor.matmul(
                        acc[:],
                        xsb[:, a, :],      # lhsT: [K=128 (i), M=128 (b)]
                        wtiles[a][:],      # rhs:  [K=128 (i), N=512 (o)]
                        start=(a == 0),
                        stop=(a == KT - 1),
                    )
                finish_tile(t, acc, split=(t == BT - 1))

    nc.compile()
    return nc


def kernel(x, weight, bias, indx_seqs):
    x = np.asarray(x, dtype=np.float32)
    weight = np.asarray(weight, dtype=np.float32)
    bias = np.asarray(bias, dtype=np.float32)
    indx_seqs = np.asarray(indx_seqs)

    if "nc" not in _cache:
        _cache["nc"] = _build()
    nc = _cache["nc"]

    # Densify sparse weights: W'[o, i] += weight[o, k] at i = indx_seqs[o, k]
    wd = np.zeros((OUT_F, IN_F), dtype=np.float32)
    np.add.at(wd, (np.arange(OUT_F)[:, None], indx_seqs), weight)

    # Host pre-tiling into SBUF-friendly layouts (cast to fp16 for the PE).
    # XT[t, p, a, c] = x[t*128+c, a*128+p]
    xt = np.ascontiguousarray(
        x.astype(np.float16).reshape(BT, P, KT, P).transpose(0, 3, 2, 1)
    ).reshape(BT, P, KT * P)
    wd16 = wd.astype(np.float16)
    in_maps = []
    for c in range(NCORES):
        wshard = wd16[c * OSH:(c + 1) * OSH]          # (512, 4096) fp16
        # WT[p, a, n] = W'[o0+n, a*128+p]
        wt = np.ascontiguousarray(
            wshard.reshape(OSH, KT, P).transpose(2, 1, 0))
        in_maps.append({
            "XT": xt,
            "WT": wt,
            "BIAS": np.ascontiguousarray(np.broadcast_to(bias[c * OSH:(c + 1) * OSH], (P, N))),
        })

    trace = bool(int(os.environ.get("BASSK_TRACE", "0"))) or bool(
        os.environ.get("BASS_TRACE"))
    if trace:
        _enable_ntff_hook()
    res = run_bass_kernel_spmd(
        nc, in_maps, list(range(NCORES)), trace=trace,
        trace_cores=list(range(NCORES)) if trace else None,
    )
    _cache["last_results"] = res

    out = np.concatenate([res.results[c]["OUT"] for c in range(NCORES)], axis=1)
    return out.astype(np.float32)


# revision 25
# speedup vs baseline: 1.0224x; 1.0180x over previous
"""Trainium2 Bass kernel for nn_LinearCondensed.

Computes out[b, o] = sum_k weight[o, k] * x[b, indx_seqs[o, k]] + bias[o]
with B=2048, IN_F=OUT_F=4096, FAN_IN=32.

Strategy: the gather has no fast on-chip primitive (any materialized gather
moves 32x the data of x itself), so we densify the sparse weight matrix on
the host -- W'[o, i] = sum_{k: indx_seqs[o,k]==i} weight[o, k] -- and run a
dense matmul out = x @ W'^T + bias on the PE array. Operands are cast to
fp16 on the host: the PE streams fp16 at the same 1 cycle/row as fp32r
(bf16 speed) but DMA traffic halves (24MB/core vs 44MB), making the kernel
PE-bound (~110us of matmul) instead of DMA-bound. fp16 products are exact
in the PE's fp22 pipeline and accumulation is fp32, so max rel err stays
at ~4e-4. OUT_F is sharded 8 ways across cores (512 columns each), x is
replicated, bias is added in the PSUM drain.

Pipeline: x loads ride the sync HWDGE queue, weights + bias ride the
scalar HW queue in parallel (gpsimd/vector queues are software-DGE at
~35GB/s), so the first matmul's dependencies (x0 k-tiles 0-3 + W' k-tile
0, ~384KB) land ~10us in, instead of waiting behind 2.5MB on one queue.
A few dummy matmuls on a zeroed scratch tile warm the PE p-state while
those DMAs are in flight. Phase 1 runs b-tiles 0..G-1 k-block-interleaved
so the PE consumes W'/x chunks as they stream in; phase 2 streams the
remaining b-tiles k-inner with x prefetched a few tiles deep. Outputs are
stored as fp16 (host upcasts; adds ~1e-4 rel err) on the scalar queue;
the final tile's drain+store is quarter-split across the scalar+sync
queues to shorten the end-of-kernel critical path. Measured ~132us vs
the ~15us of fixed framework overhead (preamble-to-first-DMA, and an
~8us all-semaphore-zeroing epilogue) + ~113us of PE-bound matmul.
"""

import os
import sys
import types

import numpy as np

import concourse.bacc as bacc
import concourse.mybir as mybir
import concourse.tile as tile
from concourse.bass_utils import run_bass_kernel_spmd

B, IN_F, OUT_F, FAN_IN = 2048, 4096, 4096, 32
NCORES = 8
OSH = OUT_F // NCORES          # 512 output features per core
P = 128                        # partitions
BT = B // P                    # 16 batch tiles
KT = IN_F // P                 # 32 contraction tiles
N = OSH                        # 512 moving columns
G = 5                          # phase-1 b-tiles (k-blocked, paced by w stream)

f32 = mybir.dt.float32
f16 = mybir.dt.float16

_cache = {}


def _enable_ntff_hook():
    """Register the ctypes NTFF profile hook (the image's antenv lacks
    axon_hooks); lets trace=True produce a neuron-profile under axon."""
    try:
        from antenv.axon_hooks import get_axon_ntff_profile_hook  # noqa: F401
        return
    except ImportError:
        pass
    try:
        import antenv
        from trn_agent_boot.trn_boot import _ntff_profile_via_ctypes

        mod = types.ModuleType("antenv.axon_hooks")
        holder = [None]
        mod.set_axon_ntff_profile_hook = lambda h: holder.__setitem__(0, h)
        mod.get_axon_ntff_profile_hook = lambda: holder[0]
        antenv.axon_hooks = mod
        sys.modules["antenv.axon_hooks"] = mod
        mod.set_axon_ntff_profile_hook(
            _ntff_profile_via_ctypes("/opt/axon/libaxon_pjrt.so"))
        import concourse.bass_utils as bu
        bu.upload_artifacts = lambda tmpdir: str(tmpdir)
    except Exception:
        pass


def _build():
    nc = bacc.Bacc()
    # Layouts (host-pretiled, all contiguous):
    #   XT[t, p, a, c] = x[t*128 + c, a*128 + p]   -> per b-tile t: [128, KT*128]
    #   WT[p, a, n]    = W'[o0 + n, a*128 + p]     -> [128, KT, 512]
    XT = nc.declare_dram_parameter("XT", [BT, P, KT * P], f16, isOutput=False)
    WT = nc.declare_dram_parameter("WT", [P, KT, N], f16, isOutput=False)
    BIAS = nc.declare_dram_parameter("BIAS", [P, N], f32, isOutput=False)
    OUT = nc.declare_dram_parameter("OUT", [B, N], f16, isOutput=True)

    XTv = XT.ap().rearrange("t p (a c) -> t p a c", a=KT)

    # k-tile chunking of the early loads: the first matmuls only need
    # x0[k 0:4] + W'[k 0:1], so those land first; later chunks are ordered
    # by the block-interleaved phase-1 consumption order.
    X_CHUNKS = [(0, 4), (4, 4), (8, 8), (16, 8), (24, 8)]
    W_CHUNKS = [(0, 1), (1, 1), (2, 2), (4, 2), (6, 2), (8, 4), (12, 4),
                (16, 4), (20, 4), (24, 4), (28, 4)]
    # phase-1 k-blocks: within a block, all G b-tiles run k-inner, so the
    # first matmuls depend on one small x chunk instead of chunk 0 of every
    # phase-1 tile (the early DMA chain is latency-bound at ~2us/chunk).
    K_BLOCKS = [(0, 4), (4, 4), (8, 8), (16, 8), (24, 8)]

    with tile.TileContext(nc) as tc:
        with (
            tc.tile_pool(name="wpool", bufs=1) as wpool,
            tc.tile_pool(name="xpool", bufs=6) as xpool,
            tc.tile_pool(name="cpool", bufs=1) as cpool,
            tc.tile_pool(name="opool", bufs=3) as opool,
            tc.tile_pool(name="psum", bufs=6, space="PSUM") as psum,
        ):
            xtiles = {}

            def alloc_x(t):
                xs = xpool.tile([P, KT, P], f16, tag="xs")
                xtiles[t] = xs
                return xs

            wsb = wpool.tile([P, KT, N], f16)

            # Phase-1 x tiles: interleave per-tile chunks on the sync queue
            # in first-use order (k-block major, t minor).
            for t in range(G):
                alloc_x(t)
            sched = []
            for t in range(G):
                for s, n in X_CHUNKS:
                    sched.append((s, t, n))
            sched.sort()
            for s, t, n in sched:
                nc.sync.dma_start(xtiles[t][:, s:s + n, :], XTv[t][:, s:s + n, :])

            # Weights + bias stream on the scalar HW queue in parallel
            # (the gpsimd/vector queues are software-DGE and crawl at
            # ~35GB/s; only sync+scalar are hardware queues).
            brow = cpool.tile([P, N], f32)
            for s, n in W_CHUNKS:
                nc.scalar.dma_start(wsb[:, s:s + n, :], WT.ap()[:, s:s + n, :])
            nc.scalar.dma_start(brow[:], BIAS[:])
            wtiles = [wsb[:, a, :] for a in range(KT)]

            # Warm-up: dummy matmuls on a zeroed scratch tile while the
            # first input DMAs are in flight (the PE would idle ~4us
            # anyway); ramps the PE out of its cold p-state so the real
            # stream starts at full clock. Results land in a scratch PSUM
            # tile that is never read.
            scratch = cpool.tile([P, N], f16)
            nc.gpsimd.memset(scratch[:], 0)
            warm = psum.tile([P, N], f32, name="warm", tag="acc")
            for _ in range(7):
                nc.tensor.matmul(warm[:], scratch[:, :P], scratch[:],
                                 start=True, stop=True)

            # bias folded into the PSUM drain: osb = acc + bias (bias row
            # pre-replicated across partitions on host). The final tile
            # splits its drain across the scalar+sync queues so the
            # end-of-kernel store is half as long.
            def finish_tile(t, acc, split=False):
                osb = opool.tile([P, N], f16, tag="osb")
                rows = OUT.ap()[t * P:(t + 1) * P, :]
                if split:
                    q = N // 4
                    for j in range(4):
                        sl = slice(j * q, (j + 1) * q)
                        nc.vector.tensor_tensor(
                            osb[:, sl], acc[:, sl], brow[:, sl],
                            mybir.AluOpType.add)
                        eng = nc.scalar if j % 2 == 0 else nc.sync
                        eng.dma_start(rows[:, sl], osb[:, sl])
                else:
                    nc.vector.tensor_tensor(
                        osb[:], acc[:], brow[:], mybir.AluOpType.add)
                    nc.scalar.dma_start(rows, osb[:])

            # Phase 1: b-tiles 0..G-1, k-block-interleaved, so the PE
            # consumes each weight/x chunk as it lands instead of idling
            # through the 4MB weight stream.
            accs = [psum.tile([P, N], f32, name=f"acc{t}", tag="acc")
                    for t in range(G)]
            for s, n in K_BLOCKS:
                for t in range(G):
                    for a in range(s, s + n):
                        nc.tensor.matmul(
                            accs[t][:], xtiles[t][:, a, :], wtiles[a][:],
                            start=(a == 0), stop=(a == KT - 1),
                        )
            for t in range(G):
                finish_tile(t, accs[t])

            # Phase 2: remaining b-tiles, k-inner, x streamed just in time.
            for t in range(G, BT):
                xsb = alloc_x(t)
                nc.sync.dma_start(xsb[:], XTv[t])
                acc = psum.tile([P, N], f32, tag="acc")
                for a in range(KT):
                    nc.tensor.matmul(
                        acc[:],
                        xsb[:, a, :],      # lhsT: [K=128 (i), M=128 (b)]
                        wtiles[a][:],      # rhs:  [K=128 (i), N=512 (o)]
                        start=(a == 0),
                        stop=(a == KT - 1),
                    )
                finish_tile(t, acc, split=(t == BT - 1))

    nc.compile()
    return nc


def kernel(x, weight, bias, indx_seqs):
    x = np.asarray(x, dtype=np.float32)
    weight = np.asarray(weight, dtype=np.float32)
    bias = np.asarray(bias, dtype=np.float32)
    indx_seqs = np.asarray(indx_seqs)

    if "nc" not in _cache:
        _cache["nc"] = _build()
    nc = _cache["nc"]

    # Densify sparse weights: W'[o, i] += weight[o, k] at i = indx_seqs[o, k]
    wd = np.zeros((OUT_F, IN_F), dtype=np.float32)
    np.add.at(wd, (np.arange(OUT_F)[:, None], indx_seqs), weight)

    # Host pre-tiling into SBUF-friendly layouts (cast to fp16 for the PE).
    # XT[t, p, a, c] = x[t*128+c, a*128+p]
    xt = np.ascontiguousarray(
        x.astype(np.float16).reshape(BT, P, KT, P).transpose(0, 3, 2, 1)
    ).reshape(BT, P, KT * P)
    wd16 = wd.astype(np.float16)
    in_maps = []
    for c in range(NCORES):
        wshard = wd16[c * OSH:(c + 1) * OSH]          # (512, 4096) fp16
        # WT[p, a, n] = W'[o0+n, a*128+p]
        wt = np.ascontiguousarray(
            wshard.reshape(OSH, KT, P).transpose(2, 1, 0))
        in_maps.append({
            "XT": xt,
            "WT": wt,
            "BIAS": np.ascontiguousarray(np.broadcast_to(bias[c * OSH:(c + 1) * OSH], (P, N))),
        })

    trace = bool(int(os.environ.get("BASSK_TRACE", "0"))) or bool(
        os.environ.get("BASS_TRACE"))
    if trace:
        _enable_ntff_hook()
    res = run_bass_kernel_spmd(
        nc, in_maps, list(range(NCORES)), trace=trace,
        trace_cores=list(range(NCORES)) if trace else None,
    )
    _cache["last_results"] = res

    out = np.concatenate([res.results[c]["OUT"] for c in range(NCORES)], axis=1)
    return out.astype(np.float32)


# revision 26
# speedup vs baseline: 1.0355x; 1.0129x over previous
"""Trainium2 Bass kernel for nn_LinearCondensed.

Computes out[b, o] = sum_k weight[o, k] * x[b, indx_seqs[o, k]] + bias[o]
with B=2048, IN_F=OUT_F=4096, FAN_IN=32.

Strategy: the gather has no fast on-chip primitive (any materialized gather
moves 32x the data of x itself), so we densify the sparse weight matrix on
the host -- W'[o, i] = sum_{k: indx_seqs[o,k]==i} weight[o, k] -- and run a
dense matmul out = x @ W'^T + bias on the PE array. Operands are cast to
fp16 on the host: the PE streams fp16 at the same 1 cycle/row as fp32r
(bf16 speed) but DMA traffic halves (24MB/core vs 44MB), making the kernel
PE-bound (~110us of matmul) instead of DMA-bound. fp16 products are exact
in the PE's fp22 pipeline and accumulation is fp32, so max rel err stays
at ~4e-4. OUT_F is sharded 8 ways across cores (512 columns each), x is
replicated, bias is added in the PSUM drain.

Pipeline: x loads ride the sync HWDGE queue, weights + bias ride the
scalar HW queue in parallel (gpsimd/vector queues are software-DGE at
~35GB/s), so the first matmul's dependencies (x0 k-tiles 0-3 + W' k-tile
0, ~384KB) land ~10us in, instead of waiting behind 2.5MB on one queue.
A few dummy matmuls on a zeroed scratch tile warm the PE p-state while
those DMAs are in flight. Phase 1 runs b-tiles 0..G-1 k-block-interleaved
so the PE consumes W'/x chunks as they stream in; phase 2 streams the
remaining b-tiles k-inner with x prefetched a few tiles deep. Outputs are
stored as fp16 (host upcasts; adds ~1e-4 rel err) on the scalar queue;
the final tile's drain+store is quarter-split across the scalar+sync
queues to shorten the end-of-kernel critical path. Measured ~132us vs
the ~15us of fixed framework overhead (preamble-to-first-DMA, and an
~8us all-semaphore-zeroing epilogue) + ~113us of PE-bound matmul.
"""

import os
import sys
import types

import numpy as np

import concourse.bacc as bacc
import concourse.mybir as mybir
import concourse.tile as tile
from concourse.bass_utils import run_bass_kernel_spmd

B, IN_F, OUT_F, FAN_IN = 2048, 4096, 4096, 32
NCORES = 8
OSH = OUT_F // NCORES          # 512 output features per core
P = 128                        # partitions
BT = B // P                    # 16 batch tiles
KT = IN_F // P                 # 32 contraction tiles
N = OSH                        # 512 moving columns
G = 5                          # phase-1 b-tiles (k-blocked, paced by w stream)

f32 = mybir.dt.float32
f16 = mybir.dt.float16

_cache = {}


def _enable_ntff_hook():
    """Register the ctypes NTFF profile hook (the image's antenv lacks
    axon_hooks); lets trace=True produce a neuron-profile under axon."""
    try:
        from antenv.axon_hooks import get_axon_ntff_profile_hook  # noqa: F401
        return
    except ImportError:
        pass
    try:
        import antenv
        from trn_agent_boot.trn_boot import _ntff_profile_via_ctypes

        mod = types.ModuleType("antenv.axon_hooks")
        holder = [None]
        mod.set_axon_ntff_profile_hook = lambda h: holder.__setitem__(0, h)
        mod.get_axon_ntff_profile_hook = lambda: holder[0]
        antenv.axon_hooks = mod
        sys.modules["antenv.axon_hooks"] = mod
        mod.set_axon_ntff_profile_hook(
            _ntff_profile_via_ctypes("/opt/axon/libaxon_pjrt.so"))
        import concourse.bass_utils as bu
        bu.upload_artifacts = lambda tmpdir: str(tmpdir)
    except Exception:
        pass


def _build():
    nc = bacc.Bacc()
    # Layouts (host-pretiled, all contiguous):
    #   XT[t, p, a, c] = x[t*128 + c, a*128 + p]   -> per b-tile t: [128, KT*128]
    #   WT[p, a, n]    = W'[o0 + n, a*128 + p]     -> [128, KT, 512]
    XT = nc.declare_dram_parameter("XT", [BT, P, KT * P], f16, isOutput=False)
    WT = nc.declare_dram_parameter("WT", [P, KT, N], f16, isOutput=False)
    BIAS = nc.declare_dram_parameter("BIAS", [P, N], f32, isOutput=False)
    OUT = nc.declare_dram_parameter("OUT", [B, N], f16, isOutput=True)

    XTv = XT.ap().rearrange("t p (a c) -> t p a c", a=KT)

    # k-tile chunking of the early loads: the first matmuls only need
    # x0[k 0:4] + W'[k 0:1], so those land first; later chunks are ordered
    # by the block-interleaved phase-1 consumption order.
    X_CHUNKS = [(0, 4), (4, 4), (8, 8), (16, 8), (24, 8)]
    W_CHUNKS = [(0, 1), (1, 1), (2, 2), (4, 2), (6, 2), (8, 4), (12, 4),
                (16, 4), (20, 4), (24, 4), (28, 4)]
    # phase-1 k-blocks: within a block, all G b-tiles run k-inner, so the
    # first matmuls depend on one small x chunk instead of chunk 0 of every
    # phase-1 tile (the early DMA chain is latency-bound at ~2us/chunk).
    K_BLOCKS = [(0, 4), (4, 4), (8, 8), (16, 8), (24, 8)]

    with tile.TileContext(nc) as tc:
        with (
            tc.tile_pool(name="wpool", bufs=1) as wpool,
            tc.tile_pool(name="xpool", bufs=6) as xpool,
            tc.tile_pool(name="cpool", bufs=1) as cpool,
            tc.tile_pool(name="opool", bufs=3) as opool,
            tc.tile_pool(name="psum", bufs=6, space="PSUM") as psum,
        ):
            xtiles = {}

            def alloc_x(t):
                xs = xpool.tile([P, KT, P], f16, tag="xs")
                xtiles[t] = xs
                return xs

            wsb = wpool.tile([P, KT, N], f16)

            # Phase-1 x tiles: interleave per-tile chunks on the sync queue
            # in first-use order (k-block major, t minor).
            for t in range(G):
                alloc_x(t)
            sched = []
            for t in range(G):
                for s, n in X_CHUNKS:
                    sched.append((s, t, n))
            sched.sort()
            for s, t, n in sched:
                nc.sync.dma_start(xtiles[t][:, s:s + n, :], XTv[t][:, s:s + n, :])

            # Weights + bias stream on the scalar HW queue in parallel
            # (the gpsimd/vector queues are software-DGE and crawl at
            # ~35GB/s; only sync+scalar are hardware queues).
            brow = cpool.tile([P, N], f32)
            for s, n in W_CHUNKS:
                nc.scalar.dma_start(wsb[:, s:s + n, :], WT.ap()[:, s:s + n, :])
            nc.scalar.dma_start(brow[:], BIAS[:])
            wtiles = [wsb[:, a, :] for a in range(KT)]

            # Warm-up: dummy matmuls on a zeroed scratch tile while the
            # first input DMAs are in flight (the PE would idle ~4us
            # anyway); ramps the PE out of its cold p-state so the real
            # stream starts at full clock. Results land in a scratch PSUM
            # tile that is never read.
            scratch = cpool.tile([P, N], f16)
            nc.gpsimd.memset(scratch[:], 0)
            warm = psum.tile([P, N], f32, name="warm", tag="acc")
            for _ in range(7):
                nc.tensor.matmul(warm[:], scratch[:, :P], scratch[:],
                                 start=True, stop=True)

            # bias folded into the PSUM drain: osb = acc + bias (bias row
            # pre-replicated across partitions on host). The final tile
            # splits its drain across the scalar+sync queues so the
            # end-of-kernel store is half as long.
            def finish_tile(t, acc, split=False):
                osb = opool.tile([P, N], f16, tag="osb")
                rows = OUT.ap()[t * P:(t + 1) * P, :]
                if split:
                    q = N // 4
                    for j in range(4):
                        sl = slice(j * q, (j + 1) * q)
                        nc.vector.tensor_tensor(
                            osb[:, sl], acc[:, sl], brow[:, sl],
                            mybir.AluOpType.add)
                        eng = nc.scalar if j % 2 == 0 else nc.sync
                        eng.dma_start(rows[:, sl], osb[:, sl])
                else:
                    nc.vector.tensor_tensor(
                        osb[:], acc[:], brow[:], mybir.AluOpType.add)
                    nc.scalar.dma_start(rows, osb[:])

            # Phase 1: b-tiles 0..G-1, k-block-interleaved, so the PE
            # consumes each weight/x chunk as it lands instead of idling
            # through the 4MB weight stream.
            accs = [psum.tile([P, N], f32, name=f"acc{t}", tag="acc")
                    for t in range(G)]
            # Staircase entry: tiles 3-4 join one k-block late, so their
            # chunk-0 x DMAs (the early bandwidth bottleneck: G chunk-0s
            # vs only 4 w k-tiles in block 0) overlap block-0 compute
            # instead of gating it.
            p1_sched = [(0, 0), (1, 0), (2, 0),
                        (0, 1), (3, 0), (1, 1), (4, 0), (2, 1), (3, 1), (4, 1)]
            p1_sched += [(t, b) for b in range(2, len(K_BLOCKS)) for t in range(G)]
            for t, b in p1_sched:
                s, n = K_BLOCKS[b]
                for a in range(s, s + n):
                    nc.tensor.matmul(
                        accs[t][:], xtiles[t][:, a, :], wtiles[a][:],
                        start=(a == 0), stop=(a == KT - 1),
                    )
            for t in range(G):
                finish_tile(t, accs[t])

            # Phase 2: remaining b-tiles, k-inner, x streamed just in time.
            for t in range(G, BT):
                xsb = alloc_x(t)
                nc.sync.dma_start(xsb[:], XTv[t])
                acc = psum.tile([P, N], f32, tag="acc")
                for a in range(KT):
                    nc.tensor.matmul(
                        acc[:],
                        xsb[:, a, :],      # lhsT: [K=128 (i), M=128 (b)]
                        wtiles[a][:],      # rhs:  [K=128 (i), N=512 (o)]
                        start=(a == 0),
                        stop=(a == KT - 1),
                    )
                finish_tile(t, acc, split=(t == BT - 1))

    nc.compile()
    return nc


def kernel(x, weight, bias, indx_seqs):
    x = np.asarray(x, dtype=np.float32)
    weight = np.asarray(weight, dtype=np.float32)
    bias = np.asarray(bias, dtype=np.float32)
    indx_seqs = np.asarray(indx_seqs)

    if "nc" not in _cache:
        _cache["nc"] = _build()
    nc = _cache["nc"]

    # Densify sparse weights: W'[o, i] += weight[o, k] at i = indx_seqs[o, k]
    wd = np.zeros((OUT_F, IN_F), dtype=np.float32)
    np.add.at(wd, (np.arange(OUT_F)[:, None], indx_seqs), weight)

    # Host pre-tiling into SBUF-friendly layouts (cast to fp16 for the PE).
    # XT[t, p, a, c] = x[t*128+c, a*128+p]
    xt = np.ascontiguousarray(
        x.astype(np.float16).reshape(BT, P, KT, P).transpose(0, 3, 2, 1)
    ).reshape(BT, P, KT * P)
    wd16 = wd.astype(np.float16)
    in_maps = []
    for c in range(NCORES):
        wshard = wd16[c * OSH:(c + 1) * OSH]          # (512, 4096) fp16
        # WT[p, a, n] = W'[o0+n, a*128+p]
        wt = np.ascontiguousarray(
            wshard.reshape(OSH, KT, P).transpose(2, 1, 0))
        in_maps.append({
            "XT": xt,
            "WT": wt,
            "BIAS": np.ascontiguousarray(np.broadcast_to(bias[c * OSH:(c + 1) * OSH], (P, N))),
        })

    trace = bool(int(os.environ.get("BASSK_TRACE", "0"))) or bool(
        os.environ.get("BASS_TRACE"))
    if trace:
        _enable_ntff_hook()
    res = run_bass_kernel_spmd(
        nc, in_maps, list(range(NCORES)), trace=trace,
        trace_cores=list(range(NCORES)) if trace else None,
    )
    _cache["last_results"] = res

    out = np.concatenate([res.results[c]["OUT"] for c in range(NCORES)], axis=1)
    return out.astype(np.float32)


# revision 27
# speedup vs baseline: 1.0360x; 1.0005x over previous
"""Trainium2 Bass kernel for nn_LinearCondensed.

Computes out[b, o] = sum_k weight[o, k] * x[b, indx_seqs[o, k]] + bias[o]
with B=2048, IN_F=OUT_F=4096, FAN_IN=32.

Strategy: the gather has no fast on-chip primitive (any materialized gather
moves 32x the data of x itself), so we densify the sparse weight matrix on
the host -- W'[o, i] = sum_{k: indx_seqs[o,k]==i} weight[o, k] -- and run a
dense matmul out = x @ W'^T + bias on the PE array. Operands are cast to
fp16 on the host: the PE streams fp16 at the same 1 cycle/row as fp32r
(bf16 speed) but DMA traffic halves (24MB/core vs 44MB), making the kernel
PE-bound (~110us of matmul) instead of DMA-bound. fp16 products are exact
in the PE's fp22 pipeline and accumulation is fp32, so max rel err stays
at ~4e-4. OUT_F is sharded 8 ways across cores (512 columns each), x is
replicated, bias is added in the PSUM drain.

Pipeline: x loads ride the sync HWDGE queue, weights + bias ride the
scalar HW queue in parallel (gpsimd/vector queues are software-DGE at
~35GB/s), so the first matmul's dependencies (x0 k-tiles 0-3 + W' k-tile
0, ~384KB) land ~10us in, instead of waiting behind 2.5MB on one queue.
A few dummy matmuls on a zeroed scratch tile warm the PE p-state while
those DMAs are in flight. Phase 1 runs b-tiles 0..G-1 k-block-interleaved
so the PE consumes W'/x chunks as they stream in; phase 2 streams the
remaining b-tiles k-inner with x prefetched a few tiles deep. Outputs are
stored as fp16 (host upcasts; adds ~1e-4 rel err) on the scalar queue;
the final tile's drain+store is quarter-split across the scalar+sync
queues to shorten the end-of-kernel critical path. Measured ~132us vs
the ~15us of fixed framework overhead (preamble-to-first-DMA, and an
~8us all-semaphore-zeroing epilogue) + ~113us of PE-bound matmul.
"""

import os
import sys
import types

import numpy as np

import concourse.bacc as bacc
import concourse.mybir as mybir
import concourse.tile as tile
from concourse.bass_utils import run_bass_kernel_spmd

B, IN_F, OUT_F, FAN_IN = 2048, 4096, 4096, 32
NCORES = 8
OSH = OUT_F // NCORES          # 512 output features per core
P = 128                        # partitions
BT = B // P                    # 16 batch tiles
KT = IN_F // P                 # 32 contraction tiles
N = OSH                        # 512 moving columns
G = 5                          # phase-1 b-tiles (k-blocked, paced by w stream)

f32 = mybir.dt.float32
f16 = mybir.dt.float16

_cache = {}


def _enable_ntff_hook():
    """Register the ctypes NTFF profile hook (the image's antenv lacks
    axon_hooks); lets trace=True produce a neuron-profile under axon."""
    try:
        from antenv.axon_hooks import get_axon_ntff_profile_hook  # noqa: F401
        return
    except ImportError:
        pass
    try:
        import antenv
        from trn_agent_boot.trn_boot import _ntff_profile_via_ctypes

        mod = types.ModuleType("antenv.axon_hooks")
        holder = [None]
        mod.set_axon_ntff_profile_hook = lambda h: holder.__setitem__(0, h)
        mod.get_axon_ntff_profile_hook = lambda: holder[0]
        antenv.axon_hooks = mod
        sys.modules["antenv.axon_hooks"] = mod
        mod.set_axon_ntff_profile_hook(
            _ntff_profile_via_ctypes("/opt/axon/libaxon_pjrt.so"))
        import concourse.bass_utils as bu
        bu.upload_artifacts = lambda tmpdir: str(tmpdir)
    except Exception:
        pass


def _build():
    nc = bacc.Bacc()
    # Layouts (host-pretiled, all contiguous):
    #   XT[t, p, a, c] = x[t*128 + c, a*128 + p]   -> per b-tile t: [128, KT*128]
    #   WT[p, a, n]    = W'[o0 + n, a*128 + p]     -> [128, KT, 512]
    XT = nc.declare_dram_parameter("XT", [BT, P, KT * P], f16, isOutput=False)
    WT = nc.declare_dram_parameter("WT", [P, KT, N], f16, isOutput=False)
    BIAS = nc.declare_dram_parameter("BIAS", [P, N], f32, isOutput=False)
    OUT = nc.declare_dram_parameter("OUT", [B, N], f16, isOutput=True)

    XTv = XT.ap().rearrange("t p (a c) -> t p a c", a=KT)

    # k-tile chunking of the early loads: the first matmuls only need
    # x0[k 0:4] + W'[k 0:1], so those land first; later chunks are ordered
    # by the block-interleaved phase-1 consumption order.
    X_CHUNKS = [(0, 4), (4, 4), (8, 8), (16, 8), (24, 8)]
    W_CHUNKS = [(0, 1), (1, 1), (2, 2), (4, 2), (6, 2), (8, 4), (12, 4),
                (16, 4), (20, 4), (24, 4), (28, 4)]
    # phase-1 k-blocks: within a block, all G b-tiles run k-inner, so the
    # first matmuls depend on one small x chunk instead of chunk 0 of every
    # phase-1 tile (the early DMA chain is latency-bound at ~2us/chunk).
    K_BLOCKS = [(0, 4), (4, 4), (8, 8), (16, 8), (24, 8)]

    with tile.TileContext(nc) as tc:
        with (
            tc.tile_pool(name="wpool", bufs=1) as wpool,
            tc.tile_pool(name="xpool", bufs=6) as xpool,
            tc.tile_pool(name="cpool", bufs=1) as cpool,
            tc.tile_pool(name="opool", bufs=3) as opool,
            tc.tile_pool(name="psum", bufs=6, space="PSUM") as psum,
        ):
            xtiles = {}

            def alloc_x(t):
                xs = xpool.tile([P, KT, P], f16, tag="xs")
                xtiles[t] = xs
                return xs

            wsb = wpool.tile([P, KT, N], f16)

            # Staircase phase-1 schedule: tiles 3-4 join one k-block late,
            # so their chunk-0 x DMAs (the early bandwidth bottleneck: G
            # chunk-0s vs only 4 w k-tiles in block 0) overlap block-0
            # compute instead of gating it. x triggers are emitted in the
            # exact consumption order.
            p1_sched = [(0, 0), (1, 0), (2, 0),
                        (0, 1), (3, 0), (1, 1), (4, 0), (2, 1), (3, 1), (4, 1)]
            p1_sched += [(t, b) for b in range(2, len(K_BLOCKS)) for t in range(G)]

            for t in range(G):
                alloc_x(t)
            for t, b in p1_sched:
                s, n = K_BLOCKS[b]
                nc.sync.dma_start(xtiles[t][:, s:s + n, :], XTv[t][:, s:s + n, :])

            # Weights + bias stream on the scalar HW queue in parallel
            # (the gpsimd/vector queues are software-DGE and crawl at
            # ~35GB/s; only sync+scalar are hardware queues).
            brow = cpool.tile([P, N], f32)
            for s, n in W_CHUNKS:
                nc.scalar.dma_start(wsb[:, s:s + n, :], WT.ap()[:, s:s + n, :])
            nc.scalar.dma_start(brow[:], BIAS[:])
            wtiles = [wsb[:, a, :] for a in range(KT)]

            # Warm-up: dummy matmuls on a zeroed scratch tile while the
            # first input DMAs are in flight (the PE would idle ~4us
            # anyway); ramps the PE out of its cold p-state so the real
            # stream starts at full clock. Results land in a scratch PSUM
            # tile that is never read.
            scratch = cpool.tile([P, N], f16)
            nc.gpsimd.memset(scratch[:], 0)
            warm = psum.tile([P, N], f32, name="warm", tag="acc")
            for _ in range(7):
                nc.tensor.matmul(warm[:], scratch[:, :P], scratch[:],
                                 start=True, stop=True)

            # bias folded into the PSUM drain: osb = acc + bias (bias row
            # pre-replicated across partitions on host). The final tile
            # splits its drain across the scalar+sync queues so the
            # end-of-kernel store is half as long.
            def finish_tile(t, acc, split=False):
                osb = opool.tile([P, N], f16, tag="osb")
                rows = OUT.ap()[t * P:(t + 1) * P, :]
                if split:
                    q = N // 4
                    for j in range(4):
                        sl = slice(j * q, (j + 1) * q)
                        nc.vector.tensor_tensor(
                            osb[:, sl], acc[:, sl], brow[:, sl],
                            mybir.AluOpType.add)
                        eng = nc.scalar if j % 2 == 0 else nc.sync
                        eng.dma_start(rows[:, sl], osb[:, sl])
                else:
                    nc.vector.tensor_tensor(
                        osb[:], acc[:], brow[:], mybir.AluOpType.add)
                    nc.scalar.dma_start(rows, osb[:])

            # Phase 1: b-tiles 0..G-1, k-block-interleaved, so the PE
            # consumes each weight/x chunk as it lands instead of idling
            # through the 4MB weight stream.
            accs = [psum.tile([P, N], f32, name=f"acc{t}", tag="acc")
                    for t in range(G)]
            for t, b in p1_sched:
                s, n = K_BLOCKS[b]
                for a in range(s, s + n):
                    nc.tensor.matmul(
                        accs[t][:], xtiles[t][:, a, :], wtiles[a][:],
                        start=(a == 0), stop=(a == KT - 1),
                    )
            for t in range(G):
                finish_tile(t, accs[t])

            # Phase 2: remaining b-tiles, k-inner, x streamed just in time.
            for t in range(G, BT):
                xsb = alloc_x(t)
                nc.sync.dma_start(xsb[:], XTv[t])
                acc = psum.tile([P, N], f32, tag="acc")
                for a in range(KT):
                    nc.tensor.matmul(
                        acc[:],
                        xsb[:, a, :],      # lhsT: [K=128 (i), M=128 (b)]
                        wtiles[a][:],      # rhs:  [K=128 (i), N=512 (o)]
                        start=(a == 0),
                        stop=(a == KT - 1),
                    )
                finish_tile(t, acc, split=(t == BT - 1))

    nc.compile()
    return nc


def kernel(x, weight, bias, indx_seqs):
    x = np.asarray(x, dtype=np.float32)
    weight = np.asarray(weight, dtype=np.float32)
    bias = np.asarray(bias, dtype=np.float32)
    indx_seqs = np.asarray(indx_seqs)

    if "nc" not in _cache:
        _cache["nc"] = _build()
    nc = _cache["nc"]

    # Densify sparse weights: W'[o, i] += weight[o, k] at i = indx_seqs[o, k]
    wd = np.zeros((OUT_F, IN_F), dtype=np.float32)
    np.add.at(wd, (np.arange(OUT_F)[:, None], indx_seqs), weight)

    # Host pre-tiling into SBUF-friendly layouts (cast to fp16 for the PE).
    # XT[t, p, a, c] = x[t*128+c, a*128+p]
    xt = np.ascontiguousarray(
        x.astype(np.float16).reshape(BT, P, KT, P).transpose(0, 3, 2, 1)
    ).reshape(BT, P, KT * P)
    wd16 = wd.astype(np.float16)
    in_maps = []
    for c in range(NCORES):
        wshard = wd16[c * OSH:(c + 1) * OSH]          # (512, 4096) fp16
        # WT[p, a, n] = W'[o0+n, a*128+p]
        wt = np.ascontiguousarray(
            wshard.reshape(OSH, KT, P).transpose(2, 1, 0))
        in_maps.append({
            "XT": xt,
            "WT": wt,
            "BIAS": np.ascontiguousarray(np.broadcast_to(bias[c * OSH:(c + 1) * OSH], (P, N))),
        })

    trace = bool(int(os.environ.get("BASSK_TRACE", "0"))) or bool(
        os.environ.get("BASS_TRACE"))
    if trace:
        _enable_ntff_hook()
    res = run_bass_kernel_spmd(
        nc, in_maps, list(range(NCORES)), trace=trace,
        trace_cores=list(range(NCORES)) if trace else None,
    )
    _cache["last_results"] = res

    out = np.concatenate([res.results[c]["OUT"] for c in range(NCORES)], axis=1)
    return out.astype(np.float32)
